# revision 5
# baseline (speedup 1.0000x reference)
"""Trainium2 Bass kernel for nn_Decoder (GRU decoder with clipped-delta
position integration).

Strategy
--------
Data-parallel over the batch N=16384: 8 cores x 2048 rows. Everything on-chip
per core runs in a *transposed* (feature-major) layout so the recurrent
matmul W_hh @ h streams h as the moving operand with weights stationary and
no per-step transposes are needed:

  h      [HID=256, 2048]  as SBUF [128, 2, 512] x4 chunks  (K-tile, batch)
  gates  [768, chunk=512] in PSUM, gate-major
  x_c    [8, 512] per chunk: rows 0-1 = prev delta (dx, dy), rows 2-6 = ctx.

Per step, per 512-column chunk:
  P1[mt<4] = W_hh[0:512] @ h + W_ih_aug @ x     (rz preact, PSUM; K=7 tail)
  P2[2]    = W_ih_aug @ x                       (i_n contribution)
  P3[2]    = W_hh[512:768] @ h                  (h_n contribution)
  r,z = sigmoid(P1 + b_rz)  -- biases ride the free per-partition ACT bias
  npre = (P2 + b_ihn) + r*(P3 + b_hhn)          -- biases via STT scalar APs
  n = tanh(npre); h = (1-z)*n + z*h on GPSIMD (Pool has no subtract/STT,
  so 1-z = (z*-1)+1 via tensor_scalar). h keeps an fp32 master copy plus a
  bf16 shadow (refreshed by one Pool copy per chunk) that feeds the PE.
  delta = W_out @ h_new  -> bias-add eviction into a spread [128,512] tile
                            (chunk c pair at partitions 32c, 32c+1).
Step-level clip: pair-sum matmul gives u' = -0.5*|d|^2/max_step^2 spread over
the same partitions; quake-seed + 2 Newton iterations on the DVE give
rsqrt(u) (no ACT table switch, all lanes busy); delta_clipped feeds pos
(+= on GPSIMD) and is written back into the x tiles by 32-aligned
DVE/Pool muls (no DMAs on the recurrence path). Output: 4 small DMAs/step.

Matmul operands are bf16 (1 col/cycle on the PE, FWL weight loads; fp32
runs at 1/4 rate and float32r trips walrus sync-wait limits); PSUM
accumulation is fp32 and the norm matmul stays fp32 for precision.
Measured on trn2 (8 axon cores): ~19 us/step -> ~1.8 ms for T=96,
absmax error ~4.6e-3 relative to absmax(reference).
"""

import sys

for _p in ("/opt/trn_rl_repo",):
    if _p not in sys.path:
        sys.path.insert(0, _p)

import numpy as np

import concourse.bass as bass
import concourse.tile as tile
from concourse.bacc import Bacc
from concourse import mybir
from concourse.bass_utils import run_bass_kernel_spmd

F32 = mybir.dt.float32
F32R = mybir.dt.float32r
BF16 = mybir.dt.bfloat16
I32 = mybir.dt.int32
AF = mybir.ActivationFunctionType
OP = mybir.AluOpType

HID = 256
CTX_DIM = 5
V_MAX = 10.1415
DT = 0.093
MS = V_MAX * DT  # max_step
N_CORES = 8
MAGIC = 0x5F3759DF - 0x400000  # quake magic adjusted for input u' = -0.5*u


def build_module(T: int, nloc: int, unroll: int = 0):
    """Trace the Bass/Tile module for one core (nloc batch columns)."""
    CH = nloc // 512  # column chunks of 512
    assert nloc % 512 == 0

    F16 = mybir.dt.float16

    nc = Bacc()

    # ---- DRAM I/O ----
    h0_d = nc.dram_tensor("h0", [2, 128, nloc], F32, kind="ExternalInput")
    h0b_d = nc.dram_tensor("h0b", [2, 128, nloc], BF16, kind="ExternalInput")
    x0_d = nc.dram_tensor("x0i", [8, nloc], BF16, kind="ExternalInput")
    pos0_d = nc.dram_tensor("pos0", [128, 512], F32, kind="ExternalInput")
    wh_d = nc.dram_tensor("wh", [2, 128, 768], BF16, kind="ExternalInput")
    wt_d = nc.dram_tensor("wt", [8, 6, 128], BF16, kind="ExternalInput")
    wo_d = nc.dram_tensor("wo", [2, 128, 2], BF16, kind="ExternalInput")
    wd2_d = nc.dram_tensor("wd2", [128, 128], F32, kind="ExternalInput")
    bv_d = nc.dram_tensor("bv", [128, 8], F32, kind="ExternalInput")
    bpk_d = nc.dram_tensor("bpk", [2, 1], F32, kind="ExternalInput")
    out_d = nc.dram_tensor("out", [T, 2 * CH, 512], F16, kind="ExternalOutput")

    with tile.TileContext(nc) as tc:
        import contextlib

        ctx = contextlib.ExitStack()
        with ctx:
            singles = ctx.enter_context(tc.tile_pool(name="singles", bufs=1))
            h_c = []
            x_c = []
            hb_c = []
            for c in range(CH):
                h_c.append(singles.tile([128, 2, 512], F32, tag=f"h{c}", name=f"h{c}"))
                x_c.append(singles.tile([8, 512], BF16, tag=f"x{c}", name=f"x{c}"))
                hb_c.append(singles.tile([128, 2, 512], BF16, tag=f"hb{c}", name=f"hb{c}"))
            pos = singles.tile([128, 512], F32, tag="pos", name="pos")
            dbtw = singles.tile([128, 512], F32, tag="dbtw", name="dbtw")
            wh = singles.tile([128, 2, 768], BF16, tag="wh", name="wh")
            wt = singles.tile([8, 6, 128], BF16, tag="wt", name="wt")
            wo = singles.tile([128, 2, 2], BF16, tag="wo", name="wo")
            wd2 = singles.tile([128, 128], F32, tag="wd2", name="wd2")
            bv = singles.tile([128, 8], F32, tag="bv", name="bv")
            bpk = singles.tile([2, 1], F32, tag="bpk", name="bpk")
            nc.vector.memset(dbtw, 0.0)

            # initial loads
            for c in range(CH):
                cs = slice(c * 512, (c + 1) * 512)
                nc.sync.dma_start(
                    out=h_c[c],
                    in_=h0_d[:, :, :].transpose([1, 0, 2])[:, :, cs])
                nc.sync.dma_start(
                    out=hb_c[c],
                    in_=h0b_d[:, :, :].transpose([1, 0, 2])[:, :, cs])
                nc.sync.dma_start(out=x_c[c], in_=x0_d[:, :][:, cs])
            nc.sync.dma_start(out=pos, in_=pos0_d[:, :])
            nc.sync.dma_start(out=wh, in_=wh_d[:, :, :].transpose([1, 0, 2]))
            nc.sync.dma_start(out=wt, in_=wt_d[:, :, :])
            nc.sync.dma_start(out=wo, in_=wo_d[:, :, :].transpose([1, 0, 2]))
            nc.sync.dma_start(out=wd2, in_=wd2_d[:, :])
            nc.sync.dma_start(out=bv, in_=bv_d[:, :])
            nc.sync.dma_start(out=bpk, in_=bpk_d[:, :])

            # pools
            pp1 = ctx.enter_context(tc.tile_pool(name="pp1", bufs=4, space="PSUM"))
            pp2 = ctx.enter_context(tc.tile_pool(name="pp2", bufs=2, space="PSUM"))
            pp3 = ctx.enter_context(tc.tile_pool(name="pp3", bufs=1, space="PSUM"))
            ppd = ctx.enter_context(tc.tile_pool(name="ppd", bufs=1, space="PSUM"))
            sb = ctx.enter_context(tc.tile_pool(name="sb", bufs=3))
            sbs = ctx.enter_context(tc.tile_pool(name="sbs", bufs=3))

            def step(t_idx):
                for c in range(CH):
                    hc = h_c[c]
                    hb = hb_c[c]
                    xc = x_c[c]
                    # --- P1: rz preactivations, 4 M-tiles ---
                    rzs = sb.tile([128, 4, 512], F32, tag="rzs", name="rzs")
                    for mt in range(4):
                        p1 = pp1.tile([128, 512], F32, tag="p1", name="p1")
                        ms_ = slice(mt * 128, (mt + 1) * 128)
                        nc.tensor.matmul(
                            p1, wh[:, 0, ms_],
                            hb[:, 0, :], start=True, stop=False)
                        nc.tensor.matmul(
                            p1, wh[:, 1, ms_],
                            hb[:, 1, :], start=False, stop=False)
                        nc.tensor.matmul(
                            p1, wt[0:7, mt, :],
                            xc[0:7, :],
                            start=False, stop=True)
                        nc.scalar.activation(
                            rzs[:, mt, :], p1, AF.Sigmoid,
                            bias=bv[:, mt:mt + 1])
                    # --- P2: i_n, P3: h_n ---
                    p2s, p3s = [], []
                    for i in range(2):
                        p2 = pp2.tile([128, 512], F32, tag="p2", name="p2")
                        nc.tensor.matmul(
                            p2, wt[0:7, 4 + i, :],
                            xc[0:7, :],
                            start=True, stop=True)
                        p2s.append(p2)
                    for i in range(2):
                        p3 = pp3.tile([128, 512], F32, tag="p3", name="p3")
                        ms_ = slice(512 + i * 128, 512 + (i + 1) * 128)
                        nc.tensor.matmul(
                            p3, wh[:, 0, ms_],
                            hb[:, 0, :], start=True, stop=False)
                        nc.tensor.matmul(
                            p3, wh[:, 1, ms_],
                            hb[:, 1, :], start=False, stop=True)
                        p3s.append(p3)
                    # --- npre = (P2 + b_ihn) + r*(P3 + b_hhn); n = tanh ---
                    npre = sb.tile([128, 2, 512], F32, tag="npre", name="npre")
                    for i in range(2):
                        t1 = sbs.tile([128, 512], F32, tag="t1", name="t1")
                        nc.vector.scalar_tensor_tensor(
                            t1, p3s[i], bv[:, 6 + i:7 + i], rzs[:, i, :],
                            op0=OP.add, op1=OP.mult)
                        nc.vector.scalar_tensor_tensor(
                            npre[:, i, :], p2s[i], bv[:, 4 + i:5 + i], t1,
                            op0=OP.add, op1=OP.add)
                    n_t = sb.tile([128, 2, 512], F32, tag="n", name="n")
                    for i in range(2):
                        nc.scalar.activation(
                            n_t[:, i, :], npre[:, i, :], AF.Tanh)
                    # --- h = n + z*(h-n): sub on DVE (Pool has no subtract),
                    # mul+f32-add on Pool, bf16 shadow add on DVE ---
                    for kt in range(2):
                        t_d = sbs.tile([128, 512], F32, tag=f"t{kt}", name=f"t{kt}")
                        nc.vector.tensor_sub(t_d, hc[:, kt, :], n_t[:, kt, :])
                        u_t = sbs.tile([128, 512], F32, tag=f"u{kt}", name=f"u{kt}")
                        nc.gpsimd.tensor_mul(u_t, rzs[:, 2 + kt, :], t_d)
                        nc.gpsimd.tensor_add(hc[:, kt, :], n_t[:, kt, :], u_t)
                        nc.vector.tensor_add(hb[:, kt, :], n_t[:, kt, :], u_t)
                    # --- delta = W_out @ h_new, spread eviction ---
                    pd = ppd.tile([2, 512], F32, tag="pdu", name="pdu")
                    nc.tensor.matmul(pd, wo[:, 0, :],
                                     hb[:, 0, :],
                                     start=True, stop=False)
                    nc.tensor.matmul(pd, wo[:, 1, :],
                                     hb[:, 1, :],
                                     start=False, stop=True)
                    nc.vector.tensor_scalar(
                        dbtw[32 * c:32 * c + 2, :], pd, bpk[0:2, :], None,
                        op0=OP.add)

                # ---- clip: s = min(MS/||delta||, 1), spread [128, 512] ----
                sqv = sbs.tile([128, 512], F32, tag="sqv", name="sqv")
                nc.gpsimd.tensor_mul(sqv, dbtw, dbtw)
                pu = ppd.tile([128, 512], F32, tag="pdu", name="pu")
                nc.tensor.matmul(pu, wd2, sqv, start=True, stop=True)
                s1i = sbs.tile([128, 512], I32, tag="s1i", name="s1i")
                nc.vector.tensor_scalar(
                    s1i, pu.bitcast(I32), 1, 0x3FFFFFFF,
                    op0=OP.logical_shift_right, op1=OP.bitwise_and)
                y0i = sbs.tile([128, 512], I32, tag="y0i", name="y0i")
                nc.vector.tensor_scalar(
                    y0i, s1i, MAGIC, -1, op0=OP.subtract, op1=OP.mult)
                y = y0i.bitcast(F32)
                ys = []
                for it in range(2):
                    m_t = sbs.tile([128, 512], F32, tag=f"m{it}", name=f"m{it}")
                    nc.vector.tensor_mul(m_t, y, y)
                    m2_t = sbs.tile([128, 512], F32, tag=f"m2{it}", name=f"m2{it}")
                    nc.vector.tensor_mul(m2_t, m_t, pu)
                    y2_t = sbs.tile([128, 512], F32, tag=f"y2{it}", name=f"y2{it}")
                    nc.vector.scalar_tensor_tensor(
                        y2_t, m2_t, 1.5, y, op0=OP.add, op1=OP.mult)
                    y = y2_t
                    ys.append(y)
                    if it == 0:
                        # x feedback tolerates 1-Newton precision (it is
                        # bf16-rounded anyway) -> unblock next step early
                        smin1 = sbs.tile([128, 512], F32, tag="smin1",
                                         name="smin1")
                        nc.vector.tensor_scalar(
                            smin1, y, 1.0, None, op0=OP.min)
                        for c in range(CH):
                            eng = nc.vector if c % 2 == 0 else nc.gpsimd
                            eng.tensor_mul(
                                x_c[c][0:2, :], smin1[32 * c:32 * c + 2, :],
                                dbtw[32 * c:32 * c + 2, :])
                # pos/output keep the 2-Newton value
                smin = sbs.tile([128, 512], F32, tag="smin", name="smin")
                nc.gpsimd.tensor_scalar(smin, y, 1.0, None, op0=OP.min)
                dct = sbs.tile([128, 512], F32, tag="dct", name="dct")
                nc.gpsimd.tensor_mul(dct, smin, dbtw)
                nc.gpsimd.tensor_add(pos, pos, dct)
                posh = sbs.tile([128, 512], F16, tag="posh", name="posh")
                nc.scalar.activation(posh, pos, AF.Copy)
                for c in range(CH):
                    nc.sync.dma_start(
                        out=out_d[t_idx, 2 * c:2 * c + 2, :],
                        in_=posh[32 * c:32 * c + 2, :])

            if unroll <= 0:
                for t in range(T):
                    step(t)
            else:
                assert T % unroll == 0
                n_iter = T // unroll
                with tc.For_i(0, n_iter * unroll, unroll) as iv:
                    for j in range(unroll):
                        step(iv + j)

    nc.finalize()
    return nc


# ---------------- host side ----------------

_module_cache: dict = {}


def _get_module(T: int, nloc: int, unroll: int):
    key = (T, nloc, unroll)
    if key not in _module_cache:
        _module_cache[key] = build_module(T, nloc, unroll)
    return _module_cache[key]


def _host_prep(inputs, nloc):
    """Build per-core in_maps from full inputs."""
    N = inputs["init_h"].shape[0]
    n_sh = N // N_CORES
    CH = nloc // 512
    W_ih = np.asarray(inputs["W_ih"], np.float32)
    W_hh = np.asarray(inputs["W_hh"], np.float32)
    b_ih = np.asarray(inputs["b_ih"], np.float32)
    b_hh = np.asarray(inputs["b_hh"], np.float32)
    W_out = np.asarray(inputs["W_out"], np.float32)
    b_out = np.asarray(inputs["b_out"], np.float32)

    import ml_dtypes
    bf16 = ml_dtypes.bfloat16
    wh = np.ascontiguousarray(W_hh.T.reshape(2, 128, 768)).astype(bf16)
    wo = np.ascontiguousarray(W_out.T.reshape(2, 128, 2)).astype(bf16)

    # K=7 input tails: rows 0-1 = delta cols of W_ih, rows 2-6 = ctx cols
    wt = np.zeros((8, 6, 128), bf16)
    for mt in range(6):
        if mt < 4:
            rows = slice(mt * 128, (mt + 1) * 128)
        else:
            rows = slice(512 + (mt - 4) * 128, 512 + (mt - 3) * 128)
        wt[0:7, mt, :] = W_ih[rows, :].T.astype(bf16)

    # biases: cols 0-3 = (b_ih+b_hh) rz tiles, 4-5 = b_ih n, 6-7 = b_hh n
    bv = np.zeros((128, 8), np.float32)
    for mt in range(4):
        bv[:, mt] = (b_ih + b_hh)[mt * 128:(mt + 1) * 128]
    for i in range(2):
        bv[:, 4 + i] = b_ih[512 + i * 128:512 + (i + 1) * 128]
        bv[:, 6 + i] = b_hh[512 + i * 128:512 + (i + 1) * 128]

    wd2 = np.zeros((128, 128), np.float32)
    for c in range(CH):
        for i in range(2):
            for j in range(2):
                wd2[32 * c + i, 32 * c + j] = -0.5 / (MS * MS)

    bpk = np.asarray(b_out, np.float32).reshape(2, 1)

    init_h = np.asarray(inputs["init_h"], np.float32)
    ctx_in = np.asarray(inputs["ctx"], np.float32)
    x0 = np.asarray(inputs["x0"], np.float32)
    y0 = np.asarray(inputs["y0"], np.float32)

    in_maps = []
    for core in range(N_CORES):
        sl = slice(core * n_sh, (core + 1) * n_sh)
        h0 = np.ascontiguousarray(init_h[sl].T.reshape(2, 128, nloc))
        h0b = h0.astype(bf16)
        x0i = np.zeros((8, nloc), bf16)
        x0i[2:7] = ctx_in[sl].T.astype(bf16)
        pos0 = np.zeros((128, 512), np.float32)
        for c in range(CH):
            pos0[32 * c + 0] = x0[sl].reshape(CH, 512)[c]
            pos0[32 * c + 1] = y0[sl].reshape(CH, 512)[c]
        in_maps.append({
            "h0": h0, "h0b": h0b, "x0i": x0i, "pos0": pos0, "wh": wh,
            "wt": wt, "wo": wo, "wd2": wd2, "bv": bv, "bpk": bpk,
        })
    return in_maps


def _host_unpack(results, T, nloc):
    CH = nloc // 512
    outs = []
    for r in results:
        arr = np.asarray(r["out"], np.float32)  # [T, 2CH, 512] rows 2c+coord
        a = arr.reshape(T, CH, 2, 512).transpose(1, 3, 0, 2)  # ch, s, T, 2
        outs.append(a.reshape(nloc, T, 2))
    return np.concatenate(outs, axis=0)


def _pick_unroll(T: int) -> int:
    for u in (4, 3, 2):
        if T % u == 0:
            return u
    return 1


def kernel(**inputs) -> np.ndarray:
    T = int(inputs["T"])
    N = inputs["init_h"].shape[0]
    nloc = N // N_CORES
    unroll = _pick_unroll(T)
    nc = _get_module(T, nloc, unroll)
    in_maps = _host_prep(inputs, nloc)
    res = run_bass_kernel_spmd(nc, in_maps, core_ids=list(range(N_CORES)))
    return _host_unpack(res.results, T, nloc)



# revision 13
# speedup vs baseline: 10.6725x; 10.6725x over previous
"""Trainium2 Bass kernel for nn_Decoder (GRU decoder with clipped-delta
position integration).

Strategy
--------
Data-parallel over the batch N=16384: 8 cores x 2048 rows. Everything on-chip
per core runs in a *transposed* (feature-major) layout so the recurrent
matmul W_hh @ h streams h as the moving operand with weights stationary and
no per-step transposes are needed:

  h      [HID=256, 2048]  as SBUF [128, 2, 512] x4 chunks  (K-tile, batch)
  gates  [768, chunk=512] in PSUM, gate-major
  x_c    [8, 512] per chunk: rows 0-1 = prev delta (dx, dy), rows 2-6 = ctx.

Per step, per 512-column chunk:
  P1[mt<4] = W_hh[0:512] @ h + W_ih_aug @ x     (rz preact, PSUM; K=7 tail)
  P2[2]    = W_ih_aug @ x                       (i_n contribution)
  P3[2]    = W_hh[512:768] @ h                  (h_n contribution)
  r,z = sigmoid(P1 + b_rz)  -- biases ride the free per-partition ACT bias
  npre = (P2 + b_ihn) + r*(P3 + b_hhn)          -- biases via STT scalar APs
  n = tanh(npre); h = (1-z)*n + z*h on GPSIMD (Pool has no subtract/STT,
  so 1-z = (z*-1)+1 via tensor_scalar). h keeps an fp32 master copy plus a
  bf16 shadow (refreshed by one Pool copy per chunk) that feeds the PE.
  delta = W_out @ h_new  -> bias-add eviction into a spread [128,512] tile
                            (chunk c pair at partitions 32c, 32c+1).
Step-level clip: pair-sum matmul gives u' = -0.5*|d|^2/max_step^2 spread over
the same partitions; quake-seed + 2 Newton iterations on the DVE give
rsqrt(u) (no ACT table switch, all lanes busy); delta_clipped feeds pos
(+= on GPSIMD) and is written back into the x tiles by 32-aligned
DVE/Pool muls (no DMAs on the recurrence path). Output: 4 small DMAs/step.

Matmul operands are bf16 (1 col/cycle on the PE, FWL weight loads; fp32
runs at 1/4 rate and float32r trips walrus sync-wait limits); PSUM
accumulation is fp32 and the norm matmul stays fp32 for precision.
Measured on trn2 (8 axon cores): ~19 us/step -> ~1.8 ms for T=96,
absmax error ~4.6e-3 relative to absmax(reference).
"""

import sys

for _p in ("/opt/trn_rl_repo",):
    if _p not in sys.path:
        sys.path.insert(0, _p)

import numpy as np

import concourse.bass as bass
import concourse.tile as tile
from concourse.bacc import Bacc
from concourse import mybir
from concourse.bass_utils import run_bass_kernel_spmd

F32 = mybir.dt.float32
F32R = mybir.dt.float32r
BF16 = mybir.dt.bfloat16
FP8 = mybir.dt.float8e4
I32 = mybir.dt.int32
AF = mybir.ActivationFunctionType
OP = mybir.AluOpType
PM = mybir.MatmulPerfMode

# fp8e4m3 path: W_hh/W_out/W_ih-tails scaled by WSCALE on host so U(-1/16,1/16)
# weights land in fp8 normal range; compensated by 1/WSCALE at PSUM eviction
# (free on the ACT scale operand; bv n-gate bias columns pre-scaled instead).
USE_FP8 = True
WSCALE = 64.0

HID = 256
CTX_DIM = 5
V_MAX = 10.1415
DT = 0.093
MS = V_MAX * DT  # max_step
N_CORES = 8
MAGIC = 0x5F3759DF - 0x400000  # quake magic adjusted for input u' = -0.5*u


def build_module(T: int, nloc: int, unroll: int = 0):
    """Trace the Bass/Tile module for one core (nloc batch columns)."""
    CH = nloc // 512  # column chunks of 512
    assert nloc % 512 == 0

    F16 = mybir.dt.float16
    HDT = FP8 if USE_FP8 else BF16

    nc = Bacc()

    # ---- DRAM I/O ----
    h0_d = nc.dram_tensor("h0", [2, 128, nloc], F32, kind="ExternalInput")
    h0b_d = nc.dram_tensor("h0b", [2, 128, nloc], HDT, kind="ExternalInput")
    x0_d = nc.dram_tensor("x0i", [8, nloc], BF16, kind="ExternalInput")
    pos0_d = nc.dram_tensor("pos0", [128, 512], F32, kind="ExternalInput")
    wh_d = nc.dram_tensor("wh", [2, 128, 768], HDT, kind="ExternalInput")
    wt_d = nc.dram_tensor("wt", [8, 6, 128], BF16, kind="ExternalInput")
    wo_d = nc.dram_tensor("wo", [2, 128, 2], HDT, kind="ExternalInput")
    wd2_d = nc.dram_tensor("wd2", [128, 128], F32, kind="ExternalInput")
    bv_d = nc.dram_tensor("bv", [128, 8], F32, kind="ExternalInput")
    bpk_d = nc.dram_tensor("bpk", [2, 1], F32, kind="ExternalInput")
    out_d = nc.dram_tensor("out", [T, 2 * CH, 512], F16, kind="ExternalOutput")

    with tile.TileContext(nc) as tc:
        import contextlib

        ctx = contextlib.ExitStack()
        with ctx:
            singles = ctx.enter_context(tc.tile_pool(name="singles", bufs=1))
            h_c = []
            x_c = []
            hb_c = []
            for c in range(CH):
                h_c.append(singles.tile([128, 2, 512], F32, tag=f"h{c}", name=f"h{c}"))
                x_c.append(singles.tile([8, 512], BF16, tag=f"x{c}", name=f"x{c}"))
                hb_c.append(singles.tile([128, 2, 512], HDT, tag=f"hb{c}", name=f"hb{c}"))
            pos = singles.tile([128, 512], F32, tag="pos", name="pos")
            dbtw = singles.tile([128, 512], F32, tag="dbtw", name="dbtw")
            wh = singles.tile([128, 2, 768], HDT, tag="wh", name="wh")
            wt = singles.tile([8, 6, 128], BF16, tag="wt", name="wt")
            wo = singles.tile([128, 2, 2], HDT, tag="wo", name="wo")
            wd2 = singles.tile([128, 128], F32, tag="wd2", name="wd2")
            bv = singles.tile([128, 8], F32, tag="bv", name="bv")
            bpk = singles.tile([2, 1], F32, tag="bpk", name="bpk")
            nc.vector.memset(dbtw, 0.0)

            # initial loads
            for c in range(CH):
                cs = slice(c * 512, (c + 1) * 512)
                nc.sync.dma_start(
                    out=h_c[c],
                    in_=h0_d[:, :, :].transpose([1, 0, 2])[:, :, cs])
                nc.sync.dma_start(
                    out=hb_c[c],
                    in_=h0b_d[:, :, :].transpose([1, 0, 2])[:, :, cs])
                nc.sync.dma_start(out=x_c[c], in_=x0_d[:, :][:, cs])
            nc.sync.dma_start(out=pos, in_=pos0_d[:, :])
            nc.sync.dma_start(out=wh, in_=wh_d[:, :, :].transpose([1, 0, 2]))
            nc.sync.dma_start(out=wt, in_=wt_d[:, :, :])
            nc.sync.dma_start(out=wo, in_=wo_d[:, :, :].transpose([1, 0, 2]))
            nc.sync.dma_start(out=wd2, in_=wd2_d[:, :])
            nc.sync.dma_start(out=bv, in_=bv_d[:, :])
            nc.sync.dma_start(out=bpk, in_=bpk_d[:, :])

            # pools
            pp1 = ctx.enter_context(tc.tile_pool(name="pp1", bufs=4, space="PSUM"))
            pp2 = ctx.enter_context(tc.tile_pool(name="pp2", bufs=2, space="PSUM"))
            pp3 = ctx.enter_context(tc.tile_pool(name="pp3", bufs=1, space="PSUM"))
            ppd = ctx.enter_context(tc.tile_pool(name="ppd", bufs=1, space="PSUM"))
            sb = ctx.enter_context(tc.tile_pool(name="sb", bufs=3))
            sbs = ctx.enter_context(tc.tile_pool(name="sbs", bufs=3))

            def step(t_idx):
                for c in range(CH):
                    hc = h_c[c]
                    hb = hb_c[c]
                    xc = x_c[c]
                    # --- P1: rz preactivations, 4 M-tiles ---
                    inv_s = (1.0 / WSCALE) if USE_FP8 else 1.0
                    rzs = sb.tile([128, 4, 512], F32, tag="rzs", name="rzs")
                    for mt in range(4):
                        p1 = pp1.tile([128, 512], F32, tag="p1", name="p1")
                        ms_ = slice(mt * 128, (mt + 1) * 128)
                        if USE_FP8:
                            nc.tensor.matmul(
                                p1, wh[:, :, ms_], hb[:, :, :],
                                start=True, stop=False, perf_mode=PM.DoubleRow)
                        else:
                            nc.tensor.matmul(
                                p1, wh[:, 0, ms_],
                                hb[:, 0, :], start=True, stop=False)
                            nc.tensor.matmul(
                                p1, wh[:, 1, ms_],
                                hb[:, 1, :], start=False, stop=False)
                        nc.tensor.matmul(
                            p1, wt[0:7, mt, :],
                            xc[0:7, :],
                            start=False, stop=True)
                        nc.scalar.activation(
                            rzs[:, mt, :], p1, AF.Sigmoid,
                            bias=bv[:, mt:mt + 1], scale=inv_s)
                    # --- P2: i_n, P3: h_n (fp8: both carry a WSCALE factor;
                    # n-gate bias columns are pre-scaled to match, tanh
                    # eviction divides back out via the ACT scale) ---
                    p2s, p3s = [], []
                    for i in range(2):
                        p2 = pp2.tile([128, 512], F32, tag="p2", name="p2")
                        nc.tensor.matmul(
                            p2, wt[0:7, 4 + i, :],
                            xc[0:7, :],
                            start=True, stop=True)
                        p2s.append(p2)
                    for i in range(2):
                        p3 = pp3.tile([128, 512], F32, tag="p3", name="p3")
                        ms_ = slice(512 + i * 128, 512 + (i + 1) * 128)
                        if USE_FP8:
                            nc.tensor.matmul(
                                p3, wh[:, :, ms_], hb[:, :, :],
                                start=True, stop=True, perf_mode=PM.DoubleRow)
                        else:
                            nc.tensor.matmul(
                                p3, wh[:, 0, ms_],
                                hb[:, 0, :], start=True, stop=False)
                            nc.tensor.matmul(
                                p3, wh[:, 1, ms_],
                                hb[:, 1, :], start=False, stop=True)
                        p3s.append(p3)
                    # --- npre = (P2 + b_ihn) + r*(P3 + b_hhn); n = tanh ---
                    npre = sb.tile([128, 2, 512], F32, tag="npre", name="npre")
                    for i in range(2):
                        t1 = sbs.tile([128, 512], F32, tag="t1", name="t1")
                        nc.vector.scalar_tensor_tensor(
                            t1, p3s[i], bv[:, 6 + i:7 + i], rzs[:, i, :],
                            op0=OP.add, op1=OP.mult)
                        nc.vector.scalar_tensor_tensor(
                            npre[:, i, :], p2s[i], bv[:, 4 + i:5 + i], t1,
                            op0=OP.add, op1=OP.add)
                    n_t = sb.tile([128, 2, 512], F32, tag="n", name="n")
                    for i in range(2):
                        nc.scalar.activation(
                            n_t[:, i, :], npre[:, i, :], AF.Tanh, scale=inv_s)
                    # --- h = n + z*(h-n): sub on DVE (Pool has no subtract),
                    # mul+f32-add on Pool, bf16 shadow add on DVE ---
                    for kt in range(2):
                        t_d = sbs.tile([128, 512], F32, tag=f"t{kt}", name=f"t{kt}")
                        nc.vector.tensor_sub(t_d, hc[:, kt, :], n_t[:, kt, :])
                        u_t = sbs.tile([128, 512], F32, tag=f"u{kt}", name=f"u{kt}")
                        nc.gpsimd.tensor_mul(u_t, rzs[:, 2 + kt, :], t_d)
                        nc.gpsimd.tensor_add(hc[:, kt, :], n_t[:, kt, :], u_t)
                        nc.vector.tensor_add(hb[:, kt, :], n_t[:, kt, :], u_t)
                    # --- delta = W_out @ h_new, spread eviction ---
                    pd = ppd.tile([2, 512], F32, tag="pdu", name="pdu")
                    if USE_FP8:
                        nc.tensor.matmul(pd, wo[:, :, :], hb[:, :, :],
                                         start=True, stop=True,
                                         perf_mode=PM.DoubleRow)
                        nc.vector.tensor_scalar(
                            dbtw[32 * c:32 * c + 2, :], pd, inv_s, bpk[0:2, :],
                            op0=OP.mult, op1=OP.add)
                    else:
                        nc.tensor.matmul(pd, wo[:, 0, :],
                                         hb[:, 0, :],
                                         start=True, stop=False)
                        nc.tensor.matmul(pd, wo[:, 1, :],
                                         hb[:, 1, :],
                                         start=False, stop=True)
                        nc.vector.tensor_scalar(
                            dbtw[32 * c:32 * c + 2, :], pd, bpk[0:2, :], None,
                            op0=OP.add)

                # ---- clip: s = min(MS/||delta||, 1), spread [128, 512] ----
                sqv = sbs.tile([128, 512], F32, tag="sqv", name="sqv")
                nc.gpsimd.tensor_mul(sqv, dbtw, dbtw)
                pu = ppd.tile([128, 512], F32, tag="pdu", name="pu")
                nc.tensor.matmul(pu, wd2, sqv, start=True, stop=True)
                s1i = sbs.tile([128, 512], I32, tag="s1i", name="s1i")
                nc.vector.tensor_scalar(
                    s1i, pu.bitcast(I32), 1, 0x3FFFFFFF,
                    op0=OP.logical_shift_right, op1=OP.bitwise_and)
                y0i = sbs.tile([128, 512], I32, tag="y0i", name="y0i")
                nc.vector.tensor_scalar(
                    y0i, s1i, MAGIC, -1, op0=OP.subtract, op1=OP.mult)
                y = y0i.bitcast(F32)
                ys = []
                for it in range(2):
                    m_t = sbs.tile([128, 512], F32, tag=f"m{it}", name=f"m{it}")
                    nc.vector.tensor_mul(m_t, y, y)
                    m2_t = sbs.tile([128, 512], F32, tag=f"m2{it}", name=f"m2{it}")
                    nc.vector.tensor_mul(m2_t, m_t, pu)
                    y2_t = sbs.tile([128, 512], F32, tag=f"y2{it}", name=f"y2{it}")
                    nc.vector.scalar_tensor_tensor(
                        y2_t, m2_t, 1.5, y, op0=OP.add, op1=OP.mult)
                    y = y2_t
                    ys.append(y)
                    if it == 0:
                        # x feedback tolerates 1-Newton precision (it is
                        # bf16-rounded anyway) -> unblock next step early
                        smin1 = sbs.tile([128, 512], F32, tag="smin1",
                                         name="smin1")
                        nc.vector.tensor_scalar(
                            smin1, y, 1.0, None, op0=OP.min)
                        for c in range(CH):
                            eng = nc.vector if c % 2 == 0 else nc.gpsimd
                            eng.tensor_mul(
                                x_c[c][0:2, :], smin1[32 * c:32 * c + 2, :],
                                dbtw[32 * c:32 * c + 2, :])
                # pos/output keep the 2-Newton value
                smin = sbs.tile([128, 512], F32, tag="smin", name="smin")
                nc.gpsimd.tensor_scalar(smin, y, 1.0, None, op0=OP.min)
                dct = sbs.tile([128, 512], F32, tag="dct", name="dct")
                nc.gpsimd.tensor_mul(dct, smin, dbtw)
                nc.gpsimd.tensor_add(pos, pos, dct)
                posh = sbs.tile([128, 512], F16, tag="posh", name="posh")
                nc.scalar.activation(posh, pos, AF.Copy)
                for c in range(CH):
                    nc.sync.dma_start(
                        out=out_d[t_idx, 2 * c:2 * c + 2, :],
                        in_=posh[32 * c:32 * c + 2, :])

            if unroll <= 0:
                for t in range(T):
                    step(t)
            else:
                assert T % unroll == 0
                n_iter = T // unroll
                with tc.For_i(0, n_iter * unroll, unroll) as iv:
                    for j in range(unroll):
                        step(iv + j)

    nc.finalize()
    return nc


# ---------------- host side ----------------

_module_cache: dict = {}


def _get_module(T: int, nloc: int, unroll: int):
    key = (T, nloc, unroll)
    if key not in _module_cache:
        _module_cache[key] = build_module(T, nloc, unroll)
    return _module_cache[key]


def _host_prep(inputs, nloc):
    """Build per-core in_maps from full inputs."""
    N = inputs["init_h"].shape[0]
    n_sh = N // N_CORES
    CH = nloc // 512
    W_ih = np.asarray(inputs["W_ih"], np.float32)
    W_hh = np.asarray(inputs["W_hh"], np.float32)
    b_ih = np.asarray(inputs["b_ih"], np.float32)
    b_hh = np.asarray(inputs["b_hh"], np.float32)
    W_out = np.asarray(inputs["W_out"], np.float32)
    b_out = np.asarray(inputs["b_out"], np.float32)

    import ml_dtypes
    bf16 = ml_dtypes.bfloat16
    ws = WSCALE if USE_FP8 else 1.0
    hdt = ml_dtypes.float8_e4m3 if USE_FP8 else bf16
    wh = np.ascontiguousarray((W_hh.T * ws).reshape(2, 128, 768)).astype(hdt)
    wo = np.ascontiguousarray((W_out.T * ws).reshape(2, 128, 2)).astype(hdt)

    # K=7 input tails: rows 0-1 = delta cols of W_ih, rows 2-6 = ctx cols
    wt = np.zeros((8, 6, 128), bf16)
    for mt in range(6):
        if mt < 4:
            rows = slice(mt * 128, (mt + 1) * 128)
        else:
            rows = slice(512 + (mt - 4) * 128, 512 + (mt - 3) * 128)
        wt[0:7, mt, :] = (W_ih[rows, :].T * ws).astype(bf16)

    # biases: cols 0-3 = (b_ih+b_hh) rz tiles (true scale, added after the
    # ACT 1/WSCALE), 4-5 = b_ih n, 6-7 = b_hh n (pre-scaled: they ride the
    # scaled PSUM values and get divided back at the tanh eviction)
    bv = np.zeros((128, 8), np.float32)
    for mt in range(4):
        bv[:, mt] = (b_ih + b_hh)[mt * 128:(mt + 1) * 128]
    for i in range(2):
        bv[:, 4 + i] = ws * b_ih[512 + i * 128:512 + (i + 1) * 128]
        bv[:, 6 + i] = ws * b_hh[512 + i * 128:512 + (i + 1) * 128]

    wd2 = np.zeros((128, 128), np.float32)
    for c in range(CH):
        for i in range(2):
            for j in range(2):
                wd2[32 * c + i, 32 * c + j] = -0.5 / (MS * MS)

    bpk = np.asarray(b_out, np.float32).reshape(2, 1)

    init_h = np.asarray(inputs["init_h"], np.float32)
    ctx_in = np.asarray(inputs["ctx"], np.float32)
    x0 = np.asarray(inputs["x0"], np.float32)
    y0 = np.asarray(inputs["y0"], np.float32)

    in_maps = []
    for core in range(N_CORES):
        sl = slice(core * n_sh, (core + 1) * n_sh)
        h0 = np.ascontiguousarray(init_h[sl].T.reshape(2, 128, nloc))
        h0b = h0.astype(hdt)
        x0i = np.zeros((8, nloc), bf16)
        x0i[2:7] = ctx_in[sl].T.astype(bf16)
        pos0 = np.zeros((128, 512), np.float32)
        for c in range(CH):
            pos0[32 * c + 0] = x0[sl].reshape(CH, 512)[c]
            pos0[32 * c + 1] = y0[sl].reshape(CH, 512)[c]
        in_maps.append({
            "h0": h0, "h0b": h0b, "x0i": x0i, "pos0": pos0, "wh": wh,
            "wt": wt, "wo": wo, "wd2": wd2, "bv": bv, "bpk": bpk,
        })
    return in_maps


def _host_unpack(results, T, nloc):
    CH = nloc // 512
    outs = []
    for r in results:
        arr = np.asarray(r["out"], np.float32)  # [T, 2CH, 512] rows 2c+coord
        a = arr.reshape(T, CH, 2, 512).transpose(1, 3, 0, 2)  # ch, s, T, 2
        outs.append(a.reshape(nloc, T, 2))
    return np.concatenate(outs, axis=0)


def _pick_unroll(T: int) -> int:
    for u in (4, 3, 2):
        if T % u == 0:
            return u
    return 1


def kernel(**inputs) -> np.ndarray:
    T = int(inputs["T"])
    N = inputs["init_h"].shape[0]
    nloc = N // N_CORES
    unroll = _pick_unroll(T)
    nc = _get_module(T, nloc, unroll)
    in_maps = _host_prep(inputs, nloc)
    res = run_bass_kernel_spmd(nc, in_maps, core_ids=list(range(N_CORES)))
    return _host_unpack(res.results, T, nloc)

